# revision 33
# baseline (speedup 1.0000x reference)
"""BitGRUCell kernel for 8 Trainium2 NeuronCores.

Strategy: data-parallel over the batch (B=4096 -> 512 rows/core), binary gate
weights replicated. Everything on device is kept feature-major ([features,
batch] = [partition, free]) so no transposes are needed anywhere:

  gate_acc[h, b] = sum_k signW.T[k, h] * actT[k, b]     (PE, fp8 DoubleRow)
  gate[h, b]     = act_fn(scale * gate_acc + bias[h])   (ScalarE, from PSUM)
  h_new          = hidden + z * (n - hidden)            (VectorE, fp32)

The matmuls run in fp8e4 (e4m3) with perf_mode=DoubleRow: the PE array holds
two +-1 weights per cell and contracts 256 k-values per pass, doubling MAC
throughput over bf16. The binary weights are exact in fp8; only the
activations are quantized (measured end-to-end rel err ~1.2e-2 vs the fp32
reference, tolerance 2e-2). The per-tensor abs-mean scale is applied via the
activation instruction's scale operand.
"""

import time

import numpy as np
import ml_dtypes

import concourse.bass as bass
import concourse.mybir as mybir
import concourse.tile as tile
from concourse import bacc
from concourse.bass import ts
from concourse.bass_utils import run_bass_kernel_spmd

B, I, H = 4096, 2048, 2048
NCORES = 8
BL = B // NCORES          # 512 batch rows per core
P = 128
KI = I // P               # 16 k-tiles in the x part
KH = H // P               # 16 k-tiles in the hidden part
KT = KI + KH              # 32 k-tiles per gate matmul
MT = H // P               # 16 output h-tiles
BF16 = mybir.dt.bfloat16
F8 = mybir.dt.float8e4
F32 = mybir.dt.float32
NPBF16 = ml_dtypes.bfloat16
NPF8 = ml_dtypes.float8_e4m3
DR = mybir.MatmulPerfMode.DoubleRow

_PROGRAM_CACHE = {}


def _build_program():
    nc = bacc.Bacc("TRN2", target_bir_lowering=False, debug=False,
                   num_devices=NCORES)

    xb_d = nc.dram_tensor("xb", [P, KI, BL], F8, kind="ExternalInput")
    hb_d = nc.dram_tensor("hb", [P, KH, BL], F8, kind="ExternalInput")
    hf_d = nc.dram_tensor("hf", [P, KH, BL], BF16, kind="ExternalInput")
    wr_d = nc.dram_tensor("wr", [MT, P, KT, P], F8, kind="ExternalInput")
    wz_d = nc.dram_tensor("wz", [MT, P, KT, P], F8, kind="ExternalInput")
    wn_d = nc.dram_tensor("wn", [MT, P, KT, P], F8, kind="ExternalInput")
    # bias columns: [b_r | b_z | b_n | -b_z]; scale: [s_r, s_z, s_n, -s_z]
    # (the negated z pair drives sigmoid(-a) = 1-z on the scalar engine)
    bias_d = nc.dram_tensor("bias", [P, 4 * MT], F32, kind="ExternalInput")
    scale_d = nc.dram_tensor("scale", [P, 4], F32, kind="ExternalInput")
    out_d = nc.dram_tensor("out", [P, MT, BL], BF16, kind="ExternalOutput")

    SIG = mybir.ActivationFunctionType.Sigmoid
    TANH = mybir.ActivationFunctionType.Tanh

    with tile.TileContext(nc) as tc:
        with (
            tc.tile_pool(name="const", bufs=1) as cpool,
            tc.tile_pool(name="persist", bufs=1) as ppool,
            tc.tile_pool(name="wpairs", bufs=4) as wppool,
            tc.tile_pool(name="acts", bufs=2) as apool,
            tc.tile_pool(name="outs", bufs=4) as opool,
            tc.tile_pool(name="psum", bufs=2, space="PSUM") as pspool,
        ):
            bias_t = cpool.tile([P, 4 * MT], F32)
            scale_t = cpool.tile([P, 4], F32)

            xb_t = ppool.tile([P, KI, BL], F8)
            hb_t = ppool.tile([P, KH, BL], F8)
            hf_t = ppool.tile([P, KH, BL], BF16)
            z_t = ppool.tile([P, MT, BL], BF16)
            rh_t = ppool.tile([P, KH, BL], F8)

            # PE warm-up: matmuls on scratch SBUF with no DMA deps. They run
            # during the initial DMA wait and trip the HAM activity monitor,
            # so the real matmuls start at 2.4 GHz instead of the cold
            # half-rate. A few small ones to get going, then 512-wide ones
            # sized to bridge until the first weight/activation chunks land
            # (~14 us) without a PE idle gap that would drop the p-state.
            warm_t = cpool.tile([P, P], BF16)
            warm_r = cpool.tile([P, BL], BF16)
            nc.vector.memset(warm_t[:], 0)
            nc.vector.memset(warm_r[:], 0)
            ps_w = pspool.tile([P, 64], F32, tag="pn", name="ps_w")
            for _ in range(24):
                nc.tensor.matmul(ps_w[:], warm_t[:], warm_t[:, :64],
                                 start=True, stop=True)
            ps_w2 = pspool.tile([P, BL], F32, tag="pn", name="ps_w2")
            for _ in range(12):
                nc.tensor.matmul(ps_w2[:], warm_t[:], warm_r[:],
                                 start=True, stop=True)

            # Weight tiles are loaded in PAIRS via one strided DMA each:
            # halves the ~650 ns Sync issue cost per dma_start and the
            # PE-side DMA-semaphore waits at matmul-group boundaries.
            def pair_dma(dst, w_dram, mp):
                # alternate the two hardware DGE rings so consecutive pair
                # loads transfer in parallel
                eng = nc.sync if (mp // 2) % 2 == 0 else nc.scalar
                eng.dma_start(
                    dst[:], w_dram[mp:mp + 2].rearrange("m p k f -> p m k f"))

            # Cold-start supply problem: spread the initial activation +
            # weight demand over the first FOUR r-tiles, k-block-interleaved.
            # Each dma_start costs ~650 ns of ISSUE time on its queue-owning
            # engine, so the cold-window loads are fanned out across three
            # otherwise-idle engines (sync: wc0, scalar: wc1, vector:
            # activations) instead of serializing ~20 issues on sync alone.
            NI = 4            # m-tiles interleaved in the cold window
            CH = 4            # k-tiles per activation-load chunk
            WCH = 8           # k-tiles per cold-weight-load chunk
            wc = [wppool.tile([P, 2, KT, P], F8, tag="wp", name=f"wc{i}")
                  for i in range(NI // 2)]
            # The two hardware DGE rings (sync, scalar) are interleaved so
            # the first matmul's two dependencies (xb chunk 0, wc0 chunk 0)
            # transfer in PARALLEL on different rings, and later chunks
            # arrive in consumption order.
            def wdma(eng, i, c):
                eng.dma_start(
                    wc[i][:, :, ts(c, WCH), :],
                    wr_d[2 * i:2 * i + 2, :, ts(c, WCH), :]
                    .rearrange("m p k f -> p m k f"))

            nc.sync.dma_start(xb_t[:, ts(0, CH), :], xb_d[:, ts(0, CH), :])
            wdma(nc.scalar, 0, 0)
            wdma(nc.sync, 1, 0)
            nc.scalar.dma_start(xb_t[:, ts(1, CH), :], xb_d[:, ts(1, CH), :])
            for c in range(1, KT // WCH):
                wdma(nc.sync, 0, c)
                wdma(nc.scalar, 1, c)
            # gpsimd (SWDGE) ring: everything consumed later.
            nc.gpsimd.dma_start(bias_t[:], bias_d[:])
            nc.gpsimd.dma_start(scale_t[:], scale_d[:])
            for c in range(2, KI // CH):
                nc.gpsimd.dma_start(xb_t[:, ts(c, CH), :],
                                    xb_d[:, ts(c, CH), :])
            for c in range(KH // CH):
                nc.gpsimd.dma_start(hb_t[:, ts(c, CH), :],
                                    hb_d[:, ts(c, CH), :])

            def rhs_for(k):
                # moving operand for k-pair starting at k (r/z gates)
                if k < KI:
                    return xb_t[:, k:k + 2, :]
                return hb_t[:, k - KI:k - KI + 2, :]

            def r_tail(m, ps):
                r_m = apool.tile([P, BL], BF16, tag="r", name="r_m")
                nc.scalar.activation(r_m[:], ps[:], SIG,
                                     bias=bias_t[:, m:m + 1],
                                     scale=scale_t[:, 0:1])
                nc.vector.tensor_mul(rh_t[:, m, :], r_m[:], hb_t[:, m, :])

            # Phase A1: r gates; rh = sigmoid(r_acc)*hidden kept in fp8.
            ps_pre = [
                pspool.tile([P, BL], F32, tag="pr", name=f"ps_pre{m}", bufs=4)
                for m in range(NI)
            ]
            for c in range(KT // WCH):
                for m in range(NI):
                    for kk in range(0, WCH, 2):
                        k = c * WCH + kk
                        nc.tensor.matmul(ps_pre[m][:],
                                         wc[m // 2][:, m % 2, k:k + 2, :],
                                         rhs_for(k),
                                         start=(k == 0), stop=(k == KT - 2),
                                         perf_mode=DR)
            for m in range(NI):
                r_tail(m, ps_pre[m])

            # hf (phase-A2/B input) trickles in across A1's warm half, three
            # tiles per weight-pair iteration, so the A2 (1-z)*h multiplies
            # never wait on DMA.
            hf_sched = {4: (0, 1, 2), 6: (3, 4, 5), 8: (6, 7, 8),
                        10: (9, 10, 11), 12: (12, 13), 14: (14, 15)}
            for mp in range(NI, MT, 2):
                wpr = wppool.tile([P, 2, KT, P], F8, tag="wp", name="wpr")
                pair_dma(wpr, wr_d, mp)
                for hm in hf_sched.get(mp, ()):
                    nc.sync.dma_start(hf_t[:, hm, :], hf_d[:, hm, :])
                for j in range(2):
                    m = mp + j
                    ps_r = pspool.tile([P, BL], F32, tag="pr", name="ps_r",
                                       bufs=4)
                    for k in range(0, KT, 2):
                        nc.tensor.matmul(ps_r[:], wpr[:, j, k:k + 2, :],
                                         rhs_for(k),
                                         start=(k == 0), stop=(k == KT - 2),
                                         perf_mode=DR)
                    r_tail(m, ps_r)

            # Phase A2: z gates. Besides z = sigmoid(a), also compute
            # 1-z = sigmoid(-a) (negated scale/bias pair) and fold
            # u = (1-z)*h into hf_t in place, so the phase-B tail chain is
            # just h_new = u + z*n (two vector ops instead of three).
            for mp in range(0, MT, 2):
                wpz = wppool.tile([P, 2, KT, P], F8, tag="wp", name="wpz")
                pair_dma(wpz, wz_d, mp)
                for j in range(2):
                    m = mp + j
                    ps_z = pspool.tile([P, BL], F32, tag="pz", name="ps_z")
                    for k in range(0, KT, 2):
                        nc.tensor.matmul(ps_z[:], wpz[:, j, k:k + 2, :],
                                         rhs_for(k),
                                         start=(k == 0), stop=(k == KT - 2),
                                         perf_mode=DR)
                    nc.scalar.activation(z_t[:, m, :], ps_z[:], SIG,
                                         bias=bias_t[:, MT + m:MT + m + 1],
                                         scale=scale_t[:, 1:2])
                    zc_m = apool.tile([P, BL], BF16, tag="zc", name="zc_m")
                    nc.scalar.activation(zc_m[:], ps_z[:], SIG,
                                         bias=bias_t[:, 3 * MT + m:3 * MT + m + 1],
                                         scale=scale_t[:, 3:4])
                    nc.vector.tensor_mul(hf_t[:, m, :], zc_m[:],
                                         hf_t[:, m, :])

            # Phase B: n gate over [x, r*hidden]; h_new = h + z*(n - h).
            # The last m-tile is processed in shrinking batch slices so its
            # activation/elementwise/store tail is short.
            for mp in range(0, MT, 2):
                wpn = wppool.tile([P, 2, KT, P], F8, tag="wp", name="wpn")
                pair_dma(wpn, wn_d, mp)
                for j in range(2):
                    m = mp + j
                    wn_m = wpn[:, j]
                    ps_n = pspool.tile([P, BL], F32, tag="pn", name="ps_n")
                    for k in range(0, KI, 2):
                        nc.tensor.matmul(ps_n[:],
                                         wn_m[:, k:k + 2, :],
                                         xb_t[:, k:k + 2, :],
                                         start=(k == 0), stop=False,
                                         perf_mode=DR)
                    for k in range(0, KH, 2):
                        nc.tensor.matmul(ps_n[:],
                                         wn_m[:, KI + k:KI + k + 2, :],
                                         rh_t[:, k:k + 2, :],
                                         start=False, stop=(k == KH - 2),
                                         perf_mode=DR)
                    n_m = apool.tile([P, BL], BF16, tag="n", name="n_m")
                    nc.scalar.activation(n_m[:], ps_n[:], TANH,
                                         bias=bias_t[:, 2 * MT + m:2 * MT + m + 1],
                                         scale=scale_t[:, 2:3])
                    d_m = apool.tile([P, BL], BF16, tag="d", name="d_m")
                    nc.vector.tensor_mul(d_m[:], z_t[:, m, :], n_m[:])
                    o_m = opool.tile([P, BL], BF16, tag="o", name="o_m")
                    nc.vector.tensor_add(o_m[:], hf_t[:, m, :], d_m[:])
                    nc.sync.dma_start(out_d[:, m, :], o_m[:])

    nc.finalize()
    return nc


def _get_program():
    if "nc" not in _PROGRAM_CACHE:
        _PROGRAM_CACHE["nc"] = _build_program()
    return _PROGRAM_CACHE["nc"]


def _prep_weight(w):
    # [H, I+H] fp32 -> sign -> fp8, tiled to [MT, P, KT, P] so that
    # wtile[m][p, k, f] = sign(w)[m*P + f, k*P + p]; each [P, KT, P] slice
    # is one contiguous 512 KB DMA whose partition dim is the contraction dim.
    s = np.sign(w).astype(NPF8)
    return np.ascontiguousarray(s.reshape(MT, P, KT, P).transpose(0, 3, 2, 1))


def _prep_act(a, dtype):
    # [BL, F] -> feature-major [P, F//P, BL]: out[p, k, b] = a[b, k*P + p]
    t = a.T.reshape(-1, P, BL).transpose(1, 0, 2)
    return np.ascontiguousarray(t).astype(dtype)


def _build_in_maps(x, hidden, w_r, b_r, w_z, b_z, w_n, b_n):
    x = np.asarray(x, np.float32)
    hidden = np.asarray(hidden, np.float32)
    w_r, w_z, w_n = (np.asarray(w, np.float32) for w in (w_r, w_z, w_n))
    b_r, b_z, b_n = (np.asarray(b, np.float32) for b in (b_r, b_z, b_n))

    wr_t, wz_t, wn_t = _prep_weight(w_r), _prep_weight(w_z), _prep_weight(w_n)
    sr, sz, sn = (np.mean(np.abs(w)) for w in (w_r, w_z, w_n))
    scales = np.array([sr, sz, sn, -sz], np.float32)
    scale_arr = np.broadcast_to(scales, (P, 4)).copy()
    bias_arr = np.concatenate(
        [b.reshape(MT, P).T for b in (b_r, b_z, b_n, -b_z)], axis=1
    ).astype(np.float32).copy()

    in_maps = []
    for c in range(NCORES):
        sl = slice(c * BL, (c + 1) * BL)
        in_maps.append({
            "xb": _prep_act(x[sl], NPF8),
            "hb": _prep_act(hidden[sl], NPF8),
            "hf": _prep_act(hidden[sl], NPBF16),
            "wr": wr_t, "wz": wz_t, "wn": wn_t,
            "bias": bias_arr, "scale": scale_arr,
        })
    return in_maps


def kernel(x, hidden, w_r, b_r, w_z, b_z, w_n, b_n):
    in_maps = _build_in_maps(x, hidden, w_r, b_r, w_z, b_z, w_n, b_n)

    nc = _get_program()
    # The axon-tunneled devices occasionally throw a transient
    # NRT_EXEC_UNIT_UNRECOVERABLE on dispatch; a retry has always succeeded.
    last_exc = None
    for _attempt in range(3):
        try:
            res = run_bass_kernel_spmd(nc, in_maps,
                                       core_ids=list(range(NCORES)))
            break
        except Exception as e:
            last_exc = e
            time.sleep(5.0)
    else:
        raise last_exc

    out = np.empty((B, H), np.float32)
    for c, r in enumerate(res.results):
        # [P, MT, BL] bf16 -> h_newT[m*P+p, b] -> [BL, H] fp32
        o = r["out"].astype(np.float32).transpose(1, 0, 2).reshape(H, BL)
        out[c * BL:(c + 1) * BL] = o.T
    return out


# revision 35
# speedup vs baseline: 1.0114x; 1.0114x over previous
"""BitGRUCell kernel for 8 Trainium2 NeuronCores.

Strategy: data-parallel over the batch (B=4096 -> 512 rows/core), binary gate
weights replicated. Everything on device is kept feature-major ([features,
batch] = [partition, free]) so no transposes are needed anywhere:

  gate_acc[h, b] = sum_k signW.T[k, h] * actT[k, b]     (PE, fp8 DoubleRow)
  gate[h, b]     = act_fn(scale * gate_acc + bias[h])   (ScalarE, from PSUM)
  h_new          = hidden + z * (n - hidden)            (VectorE, fp32)

The matmuls run in fp8e4 (e4m3) with perf_mode=DoubleRow: the PE array holds
two +-1 weights per cell and contracts 256 k-values per pass, doubling MAC
throughput over bf16. The binary weights are exact in fp8; only the
activations are quantized (measured end-to-end rel err ~1.2e-2 vs the fp32
reference, tolerance 2e-2). The per-tensor abs-mean scale is applied via the
activation instruction's scale operand.
"""

import time

import numpy as np
import ml_dtypes

import concourse.bass as bass
import concourse.mybir as mybir
import concourse.tile as tile
from concourse import bacc
from concourse.bass import ts
from concourse.bass_utils import run_bass_kernel_spmd

B, I, H = 4096, 2048, 2048
NCORES = 8
BL = B // NCORES          # 512 batch rows per core
P = 128
KI = I // P               # 16 k-tiles in the x part
KH = H // P               # 16 k-tiles in the hidden part
KT = KI + KH              # 32 k-tiles per gate matmul
MT = H // P               # 16 output h-tiles
BF16 = mybir.dt.bfloat16
F8 = mybir.dt.float8e4
F32 = mybir.dt.float32
NPBF16 = ml_dtypes.bfloat16
NPF8 = ml_dtypes.float8_e4m3
DR = mybir.MatmulPerfMode.DoubleRow

_PROGRAM_CACHE = {}


def _build_program():
    nc = bacc.Bacc("TRN2", target_bir_lowering=False, debug=False,
                   num_devices=NCORES)

    xb_d = nc.dram_tensor("xb", [P, KI, BL], F8, kind="ExternalInput")
    hb_d = nc.dram_tensor("hb", [P, KH, BL], F8, kind="ExternalInput")
    hf_d = nc.dram_tensor("hf", [P, KH, BL], BF16, kind="ExternalInput")
    wr_d = nc.dram_tensor("wr", [MT, P, KT, P], F8, kind="ExternalInput")
    wz_d = nc.dram_tensor("wz", [MT, P, KT, P], F8, kind="ExternalInput")
    wn_d = nc.dram_tensor("wn", [MT, P, KT, P], F8, kind="ExternalInput")
    # bias columns: [b_r | b_z | b_n | -b_z]; scale: [s_r, s_z, s_n, -s_z]
    # (the negated z pair drives sigmoid(-a) = 1-z on the scalar engine)
    bias_d = nc.dram_tensor("bias", [P, 4 * MT], F32, kind="ExternalInput")
    scale_d = nc.dram_tensor("scale", [P, 4], F32, kind="ExternalInput")
    out_d = nc.dram_tensor("out", [P, MT, BL], BF16, kind="ExternalOutput")

    SIG = mybir.ActivationFunctionType.Sigmoid
    TANH = mybir.ActivationFunctionType.Tanh

    with tile.TileContext(nc) as tc:
        with (
            tc.tile_pool(name="const", bufs=1) as cpool,
            tc.tile_pool(name="persist", bufs=1) as ppool,
            tc.tile_pool(name="wpairs", bufs=6) as wppool,
            tc.tile_pool(name="acts", bufs=2) as apool,
            tc.tile_pool(name="outs", bufs=4) as opool,
            tc.tile_pool(name="psum", bufs=2, space="PSUM") as pspool,
        ):
            bias_t = cpool.tile([P, 4 * MT], F32)
            scale_t = cpool.tile([P, 4], F32)

            xb_t = ppool.tile([P, KI, BL], F8)
            hb_t = ppool.tile([P, KH, BL], F8)
            hf_t = ppool.tile([P, KH, BL], BF16)
            z_t = ppool.tile([P, MT, BL], BF16)
            rh_t = ppool.tile([P, KH, BL], F8)

            # PE warm-up: matmuls on scratch SBUF with no DMA deps. They run
            # during the initial DMA wait and trip the HAM activity monitor,
            # so the real matmuls start at 2.4 GHz instead of the cold
            # half-rate. A few small ones to get going, then 512-wide ones
            # sized to bridge until the first weight/activation chunks land
            # (~14 us) without a PE idle gap that would drop the p-state.
            warm_t = cpool.tile([P, P], BF16)
            warm_r = cpool.tile([P, BL], BF16)
            nc.vector.memset(warm_t[:], 0)
            nc.vector.memset(warm_r[:], 0)
            ps_w = pspool.tile([P, 64], F32, tag="pn", name="ps_w")
            for _ in range(24):
                nc.tensor.matmul(ps_w[:], warm_t[:], warm_t[:, :64],
                                 start=True, stop=True)
            ps_w2 = pspool.tile([P, BL], F32, tag="pn", name="ps_w2")
            for _ in range(12):
                nc.tensor.matmul(ps_w2[:], warm_t[:], warm_r[:],
                                 start=True, stop=True)

            # Weight tiles are loaded in PAIRS via one strided DMA each:
            # halves the ~650 ns Sync issue cost per dma_start and the
            # PE-side DMA-semaphore waits at matmul-group boundaries.
            def pair_dma(dst, w_dram, mp):
                nc.sync.dma_start(
                    dst[:], w_dram[mp:mp + 2].rearrange("m p k f -> p m k f"))

            # Cold-start supply problem: spread the initial activation +
            # weight demand over the first FOUR r-tiles, k-block-interleaved.
            # Each dma_start costs ~650 ns of ISSUE time on its queue-owning
            # engine, so the cold-window loads are fanned out across three
            # otherwise-idle engines (sync: wc0, scalar: wc1, vector:
            # activations) instead of serializing ~20 issues on sync alone.
            NI = 4            # m-tiles interleaved in the cold window
            CH = 4            # k-tiles per activation-load chunk
            WCH = 8           # k-tiles per cold-weight-load chunk
            wc = [wppool.tile([P, 2, KT, P], F8, tag="wp", name=f"wc{i}")
                  for i in range(NI // 2)]
            # The two hardware DGE rings (sync, scalar) are interleaved so
            # the first matmul's two dependencies (xb chunk 0, wc0 chunk 0)
            # transfer in PARALLEL on different rings, and later chunks
            # arrive in consumption order.
            def wdma(eng, i, c):
                eng.dma_start(
                    wc[i][:, :, ts(c, WCH), :],
                    wr_d[2 * i:2 * i + 2, :, ts(c, WCH), :]
                    .rearrange("m p k f -> p m k f"))

            nc.sync.dma_start(xb_t[:, ts(0, CH), :], xb_d[:, ts(0, CH), :])
            wdma(nc.scalar, 0, 0)
            wdma(nc.sync, 1, 0)
            nc.scalar.dma_start(xb_t[:, ts(1, CH), :], xb_d[:, ts(1, CH), :])
            for c in range(1, KT // WCH):
                wdma(nc.sync, 0, c)
                wdma(nc.scalar, 1, c)
            # gpsimd (SWDGE) ring: everything consumed later.
            nc.gpsimd.dma_start(bias_t[:], bias_d[:])
            nc.gpsimd.dma_start(scale_t[:], scale_d[:])
            for c in range(2, KI // CH):
                nc.gpsimd.dma_start(xb_t[:, ts(c, CH), :],
                                    xb_d[:, ts(c, CH), :])
            for c in range(KH // CH):
                nc.gpsimd.dma_start(hb_t[:, ts(c, CH), :],
                                    hb_d[:, ts(c, CH), :])

            def rhs_for(k):
                # moving operand for k-pair starting at k (r/z gates)
                if k < KI:
                    return xb_t[:, k:k + 2, :]
                return hb_t[:, k - KI:k - KI + 2, :]

            def r_tail(m, ps):
                r_m = apool.tile([P, BL], BF16, tag="r", name="r_m")
                nc.scalar.activation(r_m[:], ps[:], SIG,
                                     bias=bias_t[:, m:m + 1],
                                     scale=scale_t[:, 0:1])
                nc.vector.tensor_mul(rh_t[:, m, :], r_m[:], hb_t[:, m, :])

            # Phase A1: r gates; rh = sigmoid(r_acc)*hidden kept in fp8.
            ps_pre = [
                pspool.tile([P, BL], F32, tag="pr", name=f"ps_pre{m}", bufs=4)
                for m in range(NI)
            ]
            for c in range(KT // WCH):
                for m in range(NI):
                    for kk in range(0, WCH, 2):
                        k = c * WCH + kk
                        nc.tensor.matmul(ps_pre[m][:],
                                         wc[m // 2][:, m % 2, k:k + 2, :],
                                         rhs_for(k),
                                         start=(k == 0), stop=(k == KT - 2),
                                         perf_mode=DR)
            for m in range(NI):
                r_tail(m, ps_pre[m])

            # hf (phase-A2/B input) trickles in across A1's warm half, three
            # tiles per weight-pair iteration, so the A2 (1-z)*h multiplies
            # never wait on DMA.
            hf_sched = {4: (0, 1, 2), 6: (3, 4, 5), 8: (6, 7, 8),
                        10: (9, 10, 11), 12: (12, 13), 14: (14, 15)}
            for mp in range(NI, MT, 2):
                wpr = wppool.tile([P, 2, KT, P], F8, tag="wp", name="wpr")
                pair_dma(wpr, wr_d, mp)
                for hm in hf_sched.get(mp, ()):
                    nc.sync.dma_start(hf_t[:, hm, :], hf_d[:, hm, :])
                for j in range(2):
                    m = mp + j
                    ps_r = pspool.tile([P, BL], F32, tag="pr", name="ps_r",
                                       bufs=4)
                    for k in range(0, KT, 2):
                        nc.tensor.matmul(ps_r[:], wpr[:, j, k:k + 2, :],
                                         rhs_for(k),
                                         start=(k == 0), stop=(k == KT - 2),
                                         perf_mode=DR)
                    r_tail(m, ps_r)

            # Phase A2: z gates. Besides z = sigmoid(a), also compute
            # 1-z = sigmoid(-a) (negated scale/bias pair) and fold
            # u = (1-z)*h into hf_t in place, so the phase-B tail chain is
            # just h_new = u + z*n (two vector ops instead of three).
            for mp in range(0, MT, 2):
                wpz = wppool.tile([P, 2, KT, P], F8, tag="wp", name="wpz")
                pair_dma(wpz, wz_d, mp)
                for j in range(2):
                    m = mp + j
                    ps_z = pspool.tile([P, BL], F32, tag="pz", name="ps_z")
                    for k in range(0, KT, 2):
                        nc.tensor.matmul(ps_z[:], wpz[:, j, k:k + 2, :],
                                         rhs_for(k),
                                         start=(k == 0), stop=(k == KT - 2),
                                         perf_mode=DR)
                    nc.scalar.activation(z_t[:, m, :], ps_z[:], SIG,
                                         bias=bias_t[:, MT + m:MT + m + 1],
                                         scale=scale_t[:, 1:2])
                    zc_m = apool.tile([P, BL], BF16, tag="zc", name="zc_m")
                    nc.scalar.activation(zc_m[:], ps_z[:], SIG,
                                         bias=bias_t[:, 3 * MT + m:3 * MT + m + 1],
                                         scale=scale_t[:, 3:4])
                    nc.vector.tensor_mul(hf_t[:, m, :], zc_m[:],
                                         hf_t[:, m, :])

            # Phase B: n gate over [x, r*hidden]; h_new = h + z*(n - h).
            # The last m-tile is processed in shrinking batch slices so its
            # activation/elementwise/store tail is short.
            for mp in range(0, MT, 2):
                wpn = wppool.tile([P, 2, KT, P], F8, tag="wp", name="wpn")
                pair_dma(wpn, wn_d, mp)
                for j in range(2):
                    m = mp + j
                    wn_m = wpn[:, j]
                    ps_n = pspool.tile([P, BL], F32, tag="pn", name="ps_n")
                    for k in range(0, KI, 2):
                        nc.tensor.matmul(ps_n[:],
                                         wn_m[:, k:k + 2, :],
                                         xb_t[:, k:k + 2, :],
                                         start=(k == 0), stop=False,
                                         perf_mode=DR)
                    for k in range(0, KH, 2):
                        nc.tensor.matmul(ps_n[:],
                                         wn_m[:, KI + k:KI + k + 2, :],
                                         rh_t[:, k:k + 2, :],
                                         start=False, stop=(k == KH - 2),
                                         perf_mode=DR)
                    n_m = apool.tile([P, BL], BF16, tag="n", name="n_m")
                    nc.scalar.activation(n_m[:], ps_n[:], TANH,
                                         bias=bias_t[:, 2 * MT + m:2 * MT + m + 1],
                                         scale=scale_t[:, 2:3])
                    d_m = apool.tile([P, BL], BF16, tag="d", name="d_m")
                    nc.vector.tensor_mul(d_m[:], z_t[:, m, :], n_m[:])
                    o_m = opool.tile([P, BL], BF16, tag="o", name="o_m")
                    nc.vector.tensor_add(o_m[:], hf_t[:, m, :], d_m[:])
                    nc.sync.dma_start(out_d[:, m, :], o_m[:])

    nc.finalize()
    return nc


def _get_program():
    if "nc" not in _PROGRAM_CACHE:
        _PROGRAM_CACHE["nc"] = _build_program()
    return _PROGRAM_CACHE["nc"]


def _prep_weight(w):
    # [H, I+H] fp32 -> sign -> fp8, tiled to [MT, P, KT, P] so that
    # wtile[m][p, k, f] = sign(w)[m*P + f, k*P + p]; each [P, KT, P] slice
    # is one contiguous 512 KB DMA whose partition dim is the contraction dim.
    s = np.sign(w).astype(NPF8)
    return np.ascontiguousarray(s.reshape(MT, P, KT, P).transpose(0, 3, 2, 1))


def _prep_act(a, dtype):
    # [BL, F] -> feature-major [P, F//P, BL]: out[p, k, b] = a[b, k*P + p]
    t = a.T.reshape(-1, P, BL).transpose(1, 0, 2)
    return np.ascontiguousarray(t).astype(dtype)


def _build_in_maps(x, hidden, w_r, b_r, w_z, b_z, w_n, b_n):
    x = np.asarray(x, np.float32)
    hidden = np.asarray(hidden, np.float32)
    w_r, w_z, w_n = (np.asarray(w, np.float32) for w in (w_r, w_z, w_n))
    b_r, b_z, b_n = (np.asarray(b, np.float32) for b in (b_r, b_z, b_n))

    wr_t, wz_t, wn_t = _prep_weight(w_r), _prep_weight(w_z), _prep_weight(w_n)
    sr, sz, sn = (np.mean(np.abs(w)) for w in (w_r, w_z, w_n))
    scales = np.array([sr, sz, sn, -sz], np.float32)
    scale_arr = np.broadcast_to(scales, (P, 4)).copy()
    bias_arr = np.concatenate(
        [b.reshape(MT, P).T for b in (b_r, b_z, b_n, -b_z)], axis=1
    ).astype(np.float32).copy()

    in_maps = []
    for c in range(NCORES):
        sl = slice(c * BL, (c + 1) * BL)
        in_maps.append({
            "xb": _prep_act(x[sl], NPF8),
            "hb": _prep_act(hidden[sl], NPF8),
            "hf": _prep_act(hidden[sl], NPBF16),
            "wr": wr_t, "wz": wz_t, "wn": wn_t,
            "bias": bias_arr, "scale": scale_arr,
        })
    return in_maps


def kernel(x, hidden, w_r, b_r, w_z, b_z, w_n, b_n):
    in_maps = _build_in_maps(x, hidden, w_r, b_r, w_z, b_z, w_n, b_n)

    nc = _get_program()
    # The axon-tunneled devices occasionally throw a transient
    # NRT_EXEC_UNIT_UNRECOVERABLE on dispatch; a retry has always succeeded.
    last_exc = None
    for _attempt in range(3):
        try:
            res = run_bass_kernel_spmd(nc, in_maps,
                                       core_ids=list(range(NCORES)))
            break
        except Exception as e:
            last_exc = e
            time.sleep(5.0)
    else:
        raise last_exc

    out = np.empty((B, H), np.float32)
    for c, r in enumerate(res.results):
        # [P, MT, BL] bf16 -> h_newT[m*P+p, b] -> [BL, H] fp32
        o = r["out"].astype(np.float32).transpose(1, 0, 2).reshape(H, BL)
        out[c * BL:(c + 1) * BL] = o.T
    return out
